# revision 34
# baseline (speedup 1.0000x reference)
"""Causal single-head attention layer on 8 TRN2 NeuronCores.

Reference (per batch b):
  Q = x@Wq+bq; K = x@Wk+bk; V = x@Wv+bv        (S=4096, D=512, H=64)
  S = Q K^T / sqrt(S);  P = softmax(S + causal_mask);  out = (P V) @ Wo + bo

Sharding: 8 cores = 4 batches x 2 halves. Each core owns 4 query-blocks
of 512 rows of its batch in ASCENDING causal order: even cores take
blocks [0,3,4,7], odd take [1,2,5,6]. SPMD structural k-tile counts per
slot NKT=[8,16,24,32] cover both parities; over-structural k-tiles and
the causal boundary are killed by a multiplicative {0,1} mask generated
ON-CHIP from an fp16 iota ramp compared against per-core thresholds.

QK PE-array row-tiling (the big PE win vs the unpaired version): the QK
matmul contracts over H=64, which uses only half the 128-row array.
kvt alternates layout per 512-col half-chunk (even: V^T rows 0:64 /
K^T rows 64:128; odd: K^T 0:64 / V^T 64:128) and Q^T is duplicated to
both partition halves (the Q-projection pairs two M=64 matmuls on
disjoint column strips, so duplication costs no extra PE passes). Each
score group pairs one "hi" k-tile (8c+j) with one "lo" k-tile (8c+4+j);
the two QK matmuls land on disjoint 64-row groups (auto tile_position)
and execute CONCURRENTLY, near-halving QK PE time (269 -> 177 ns/MM
measured). The iota pattern [[-128,4],[-512,2],[1,512]] matches the
permuted (hi,lo) tail column order so one threshold per slot masks the
causal boundary. (An analogous split of the AV matmul into concurrent
K=64 halves measured SLOWER - more MM issues + psum merge work - and
column-tiled M=65 packing cannot host the ones-column denominator, so
AV stays one M=65 matmul per k-tile.)

Per group: S^T [128k, 1024] via the concurrent QK pair -> exp via ACT
(scale=1/64 folded) -> P^T bf16 -> multiplicative {0,1} mask (tail
groups only, DVE) -> AV accumulate otp[65,512] (V_aug carries a ones
column so the softmax denominator falls out of row 64; V natural layout
via XBAR transpose DMAs, source partition half alternating with the
kvt layout). QK(g+1) is emitted before AV(g) so the PE never waits on
the exp. Epilogue: all four denominator K=1 matmuls land in columns
0:4 of one psum tile, ONE reciprocal serves the slot (psum-buffer
churn between dn and y matmuls previously spaced the tail y-matmuls
~2us apart), y = ot^T @ [Wo; bv@Wo+bo] scaled by 1/denom -> bf16 out
DMA (host casts to f32). NOTE: skip_group_check=True on interleaved
same-tile accumulation chains mis-lowers start flags (NaN) - only the
Q-projection pair needs and tolerates it.

DMA schedule (measured ~150 GB/s aggregate effective during the
multi-queue lead-in; HWDGE rings are FIFO per issuing engine and a
descriptor is processed only after all prior ring entries fully
complete, ~2us HBM receipt each):
 - sync ring: fpack-equivalents ride scalar; xt[0:1024] as two 4-j-tile
   merged DMAs (a [128,4,512] AP view of xt^T - one dma_start each,
   ~600ns issue cost per dma_start on the sequencer), then vtr0, whose
   engine-block doubles as the GATE: the bulk loads emitted after it
   enter the rings only once the first wave has drained;
 - scalar ring: KV weights, xt[512:1024], Q weights, thr, wo - light,
   so the scalar engine reaches vtr-free exps early;
 - gpsimd SWDGE only carries slack-rich late loads (iota first, then
   xtq blocks 2-3, xt cols 2048:4096) - SWDGE descgen is slow (~0.6-
   1.5us/DMA) and its ring blocks the Q7, so nothing deadline-critical
   rides it. Slot 0-2 output DMAs are deferred and issued on sync just
   before slot 3; slot 3's fan out across scalar+sync.
 - PE p-state warmup (~3.4us HAM window at 1.2 GHz): 8 dependency-free
   matmuls burn the DMA lead-in so real matmuls run at 2.4 GHz.
"""

import os

os.environ.setdefault("MYCRO_LOCAL_CACHE", "1")

import numpy as np
import ml_dtypes

import concourse.mybir as mybir
import concourse.tile as tile
from concourse import bacc
from concourse.bass_utils import run_bass_kernel_spmd

F32 = mybir.dt.float32
F16 = mybir.dt.float16
BF16 = mybir.dt.bfloat16

B, S, D, H = 4, 4096, 512, 64
QB = 512
NKT = [8, 16, 24, 32]          # structural k-tiles per slot (ascending)
BLOCKS_EVEN = [0, 3, 4, 7]
BLOCKS_ODD = [1, 2, 5, 6]

LAST_EXEC_TIME_NS = None
LAST_RESULTS = None


def _install_ntff_hook():
    import sys
    import types
    try:
        from antenv.axon_hooks import get_axon_ntff_profile_hook  # noqa: F401
        return True
    except ImportError:
        pass
    try:
        import trn_agent_boot.trn_boot as _tb
        hook = _tb._ntff_profile_via_ctypes("/opt/axon/libaxon_pjrt.so")
        if hook is None:
            return False
        mod = types.ModuleType("antenv.axon_hooks")
        mod.get_axon_ntff_profile_hook = lambda: hook
        mod.set_axon_ntff_profile_hook = lambda h: None
        sys.modules["antenv.axon_hooks"] = mod
        return True
    except Exception:
        return False


def _build_nc():
    nc = bacc.Bacc(
        "TRN2",
        target_bir_lowering=False,
        debug=False,
        enable_asserts=False,
        num_devices=8,
    )

    xt_d = nc.dram_tensor("xt", [D, S], BF16, kind="ExternalInput")
    xtq_d = nc.dram_tensor("xtq", [D, 4 * QB], BF16, kind="ExternalInput")
    wpack_d = nc.dram_tensor("wpack", [128, 1280], BF16, kind="ExternalInput")
    wo_d = nc.dram_tensor("wo", [H + 1, D], BF16, kind="ExternalInput")
    fpack_d = nc.dram_tensor("fpack", [128, 4], F32, kind="ExternalInput")
    thr_d = nc.dram_tensor("thr", [128, 16], F32, kind="ExternalInput")
    out_d = nc.dram_tensor("out", [4 * QB, D], BF16, kind="ExternalOutput")

    with tile.TileContext(nc) as tc:
        with (
            tc.tile_pool(name="big", bufs=1) as big,
            tc.tile_pool(name="small", bufs=1) as small,
            tc.tile_pool(name="projps", bufs=2, space="PSUM") as projps,
            tc.tile_pool(name="stps", bufs=2, space="PSUM") as stps,
            tc.tile_pool(name="otps", bufs=2, space="PSUM") as otps,
            tc.tile_pool(name="ptp", bufs=4) as ptp,
            tc.tile_pool(name="epi", bufs=2) as epi,
        ):
            # ---- persistent SBUF ----
            xt4_sb = big.tile([128, 4, S], BF16, tag="xt4")
            xtq4_sb = big.tile([128, 4, 4 * QB], BF16, tag="xtq4")
            xt4_in = xt_d[:].rearrange("(j p) s -> p j s", p=128)
            xtq4_in = xtq_d[:].rearrange("(j p) s -> p j s", p=128)
            kvt_sb = big.tile([128, S], BF16, tag="kvt")      # per half-chunk: even V^T|K^T, odd K^T|V^T
            qtp_sb = big.tile([128, 4 * QB], BF16, tag="qtp")  # Q^T duplicated on both halves
            vaug_sb = big.tile([128, 32 * 80], BF16, tag="vaug")
            iota_sb = big.tile([128, 4, 2, 512], F16, tag="iota")
            mask_sb = big.tile([128, 4, 4096], BF16, tag="mask")
            wpack_sb = small.tile([128, 1280], BF16, tag="wpack")
            wo_sb = small.tile([H + 1, D], BF16, tag="wo")
            fpack_sb = small.tile([128, 4], F32, tag="fpack")
            thr_sb = small.tile([128, 16], F32, tag="thr")
            ones_sb = small.tile([128, 1], BF16, tag="ones")

            vaug3 = vaug_sb[:].rearrange("p (k c) -> p k c", c=80)
            bkv_hi = fpack_sb[:, 0:1]   # [0;bk]  (K^T on rows 64:128)
            bkv_lo = fpack_sb[:, 1:2]   # [bk;0]  (K^T on rows 0:64)
            bq_ap = fpack_sb[:, 2:3]    # bq duplicated both halves

            # ---- input DMAs ----
            # scalar HWDGE queue: KV weights, xt cols 512:1024, Q weights -
            # light, so the scalar engine reaches its first exp early.
            # sync: the critical first wave; the engine-blocking vtr0/vtr1
            # transposes act as natural gates for the bulk loads behind them.
            # gpsimd SWDGE: iota, then slack-rich late loads.
            nc.scalar.dma_start(out=wpack_sb[:, 0:1024], in_=wpack_d[:, 0:1024])
            nc.scalar.dma_start(out=fpack_sb[:], in_=fpack_d[:, :])
            nc.sync.dma_start(out=xt4_sb[:, :, 0:512], in_=xt4_in[:, :, 0:512])
            nc.sync.dma_start(out=xtq4_sb[:, :, 0:512], in_=xtq4_in[:, :, 0:512])
            nc.scalar.dma_start(out=xt4_sb[:, :, 512:1024], in_=xt4_in[:, :, 512:1024])
            nc.scalar.dma_start(out=wpack_sb[:, 1024:1280], in_=wpack_d[:, 1024:1280])
            nc.scalar.dma_start(out=thr_sb[:], in_=thr_d[:, :])
            nc.scalar.dma_start(out=wo_sb[:], in_=wo_d[:, :])
            # iota ramp v[p,a,b,j] = -128a - 512b + j - p  (f16 exact, |v|<2048)
            # column order matches (group, hi/lo, q) so kt_rel = a + 4b
            nc.gpsimd.iota(
                iota_sb[:], pattern=[[-128, 4], [-512, 2], [1, 512]], base=0,
                channel_multiplier=-1, allow_small_or_imprecise_dtypes=True,
            )

            def emit_xt(c0, c1, eng):
                eng.dma_start(out=xt4_sb[:, :, c0:c1], in_=xt4_in[:, :, c0:c1])

            def emit_xtq(c0, c1, eng):
                eng.dma_start(out=xtq4_sb[:, :, c0:c1], in_=xtq4_in[:, :, c0:c1])

            # PE p-state warmup: dependency-free matmuls burn the DMA lead-in
            warm_sb = small.tile([128, 512], BF16, tag="warm")
            nc.vector.memset(warm_sb[:], 0.0)

            def emit_warm(n):
                # one psum tile for all warm matmuls (they serialize on
                # has_written, which is fine - the point is PE activity)
                wp = projps.tile([128, 512], F32, tag="proj")
                for _ in range(n):
                    nc.tensor.matmul(
                        wp[:], lhsT=warm_sb[:, 0:128], rhs=warm_sb[:],
                        start=True, stop=True,
                    )

            emit_warm(9)
            nc.vector.memset(vaug3[:, :, 64:65], 1.0)
            nc.vector.memset(ones_sb[:], 1.0)

            def emit_mask(s):
                # keep-mask[p,c] = 1.0 where iota >= thr[s] else 0  (f16 in, bf16 out)
                nc.vector.tensor_scalar(
                    out=mask_sb[:, s, :],
                    in0=iota_sb[:].rearrange("p a b c -> p (a b c)"),
                    scalar1=thr_sb[:, 4 * s:4 * s + 1],
                    scalar2=None,
                    op0=mybir.AluOpType.is_ge,
                )

            def emit_kv_chunk(c):
                # seq cols [c*1024,(c+1)*1024) = k-tiles 8c..8c+7
                # half-chunk h=2c+half: h even -> [V^T;K^T] (wpack A), h odd -> [K^T;V^T] (B)
                for half in range(2):
                    h = 2 * c + half
                    col = c * 1024 + half * 512
                    woff = 0 if h % 2 == 0 else 512
                    kvp = projps.tile([128, 512], F32, tag="proj")
                    for j in range(4):
                        nc.tensor.matmul(
                            kvp[:],
                            lhsT=wpack_sb[:, woff + j * 128:woff + (j + 1) * 128],
                            rhs=xt4_sb[:, j, col:col + 512],
                            start=(j == 0),
                            stop=(j == 3),
                        )
                    nc.vector.tensor_scalar_add(
                        kvt_sb[:, col:col + 512], kvp[:],
                        bkv_hi if h % 2 == 0 else bkv_lo,
                    )

            def emit_vtr(c, eng):
                # V^T -> V natural via the XBAR transpose DMA, per 512-col piece.
                # V^T partition half alternates with the kvt layout.
                for p in range(2):
                    h = 2 * c + p
                    col = c * 1024 + p * 512
                    kt0 = 4 * h
                    src = kvt_sb[0:64, col:col + 512] if h % 2 == 0 \
                        else kvt_sb[64:128, col:col + 512]
                    eng.dma_start_transpose(
                        out=vaug3[:, kt0:kt0 + 4, 0:64], in_=src,
                    )

            def emit_q_block(blk):
                # Q^T duplicated to BOTH partition halves: two M=64 matmuls on
                # disjoint column strips run concurrently (col tiling) - the
                # duplication is free. lo -> qp[0:64], hi -> qp[64:128].
                qp = projps.tile([128, 512], F32, tag="proj")
                for j in range(4):
                    for lo in (True, False):
                        nc.tensor.matmul(
                            qp[0:64, :] if lo else qp[64:128, :],
                            lhsT=wpack_sb[:, 1024 + j * H:1024 + (j + 1) * H],
                            rhs=xtq4_sb[:, j, blk * 512:(blk + 1) * 512],
                            start=(j == 0),
                            stop=(j == 3),
                            skip_group_check=True,
                        )
                nc.vector.tensor_scalar_add(
                    qtp_sb[:, blk * 512:(blk + 1) * 512], qp[:], bq_ap
                )

            def emit_slot(s):
                nkt = NKT[s]
                ngrp = nkt // 2
                otp = otps.tile([H + 1, 512], F32, tag="otp")
                pts = {}

                def pair(g):
                    c8, j = divmod(g, 4)
                    return 8 * c8 + j, 8 * c8 + 4 + j  # (hi kt, lo kt)

                for g in range(ngrp + 1):
                    if g < ngrp:
                        ktH, ktL = pair(g)
                        stp = stps.tile([128, 1024], F32, tag="stp")
                        # concurrent row-tiled QK pair (disjoint 64-row groups)
                        nc.tensor.matmul(
                            stp[:, 0:512],
                            lhsT=kvt_sb[64:128, ktH * 128:(ktH + 1) * 128],
                            rhs=qtp_sb[64:128, s * 512:(s + 1) * 512],
                            start=True, stop=True,
                        )
                        nc.tensor.matmul(
                            stp[:, 512:1024],
                            lhsT=kvt_sb[0:64, ktL * 128:(ktL + 1) * 128],
                            rhs=qtp_sb[0:64, s * 512:(s + 1) * 512],
                            start=True, stop=True,
                        )
                        pt = ptp.tile([128, 1024], BF16, tag="pt")
                        nc.scalar.activation(
                            pt[:], stp[:], mybir.ActivationFunctionType.Exp,
                            scale=1.0 / 64.0,
                        )
                        if g >= ngrp - 4:
                            gm = g - (ngrp - 4)
                            nc.vector.tensor_mul(
                                pt[:],
                                pt[:],
                                mask_sb[:, s, gm * 1024:(gm + 1) * 1024],
                            )
                        pts[g] = pt
                    if g >= 1:
                        ptm = pts.pop(g - 1)
                        ktH, ktL = pair(g - 1)
                        for u, kt in ((0, ktH), (1, ktL)):
                            nc.tensor.matmul(
                                otp[:],
                                lhsT=vaug3[:, kt, 0:65],
                                rhs=ptm[:, u * 512:(u + 1) * 512],
                                start=(g == 1 and u == 0),
                                stop=(g == ngrp and u == 1),
                            )
                # epilogue A: stash ot (incl. denominator row 64) in bf16
                ot_sb = epi.tile([H + 1, 512], BF16, tag="ot_sb")
                dnrow = epi.tile([1, 512], BF16, tag="dnrow")
                if s == 3:
                    # kernel tail: halve the chain across both idle engines
                    nc.scalar.copy(dnrow[:], otp[H:H + 1, :])
                    nc.vector.tensor_copy(ot_sb[:, 0:256], otp[:, 0:256])
                    nc.scalar.copy(ot_sb[:, 256:512], otp[:, 256:512])
                else:
                    nc.vector.tensor_copy(ot_sb[:], otp[:])
                    nc.vector.tensor_copy(dnrow[:], otp[H:H + 1, :])
                return ot_sb, dnrow

            deferred_outs = []

            def emit_epi_b(s, ot_sb, dnrow):
                # the final slot's epilogue is the kernel tail: fan its scale
                # and output DMA across two engines/queues each. All four
                # denominator matmuls land in one psum tile (column per t) so
                # a single reciprocal serves the whole slot and the y matmuls
                # don't churn psum buffers against the dn chain.
                last = s == 3
                dnp = projps.tile([128, 512], F32, tag="proj")
                for t in range(4):
                    nc.tensor.matmul(
                        dnp[:, t:t + 1],
                        lhsT=dnrow[:, t * 128:(t + 1) * 128],
                        rhs=ones_sb[0:1, :],
                        start=True,
                        stop=True,
                    )
                recip = epi.tile([128, 4], F32, tag="recip")
                nc.vector.reciprocal(recip[:], dnp[:, 0:4])
                for t in range(4):
                    yp = projps.tile([128, 512], F32, tag="proj")
                    nc.tensor.matmul(
                        yp[:],
                        lhsT=ot_sb[:, t * 128:(t + 1) * 128],
                        rhs=wo_sb[:],
                        start=True,
                        stop=True,
                    )
                    ysb = epi.tile([128, 512], BF16, tag="ysb", bufs=14)
                    if last and t % 2 == 1:
                        nc.scalar.activation(
                            ysb[:], yp[:], mybir.ActivationFunctionType.Copy,
                            scale=recip[:, t:t + 1],
                        )
                    else:
                        nc.vector.tensor_scalar_mul(
                            ysb[:], yp[:], recip[:, t:t + 1]
                        )
                    if last:
                        out_eng = nc.scalar if t % 2 == 1 else nc.sync
                        out_eng.dma_start(
                            out=out_d[s * 512 + t * 128:s * 512 + (t + 1) * 128, :],
                            in_=ysb[:],
                        )
                    else:
                        # defer the out DMA: issuing it now would block the
                        # sync engine on the ysb semaphore while load DMAs
                        # still need issuing
                        deferred_outs.append((s, t, ysb))

            emit_kv_chunk(0)
            # vtr0 on sync doubles as the bulk-load gate: the sync engine
            # blocks here until kvt h0/h1 exist, so everything emitted after
            # enters the DMA rings only once the first wave has drained
            emit_vtr(0, nc.sync)
            emit_xt(1024, 2048, nc.sync)
            emit_xtq(512, 1024, nc.sync)
            emit_xtq(1024, 2048, nc.gpsimd)
            emit_xt(2048, 3072, nc.gpsimd)
            emit_xt(3072, 4096, nc.gpsimd)
            emit_q_block(0)
            emit_mask(0)
            emit_mask(1)
            ot0 = emit_slot(0)
            emit_q_block(1)
            emit_kv_chunk(1)
            emit_vtr(1, nc.sync)
            emit_mask(2)
            emit_mask(3)
            emit_epi_b(0, *ot0)
            ot1 = emit_slot(1)
            emit_q_block(2)
            emit_q_block(3)
            emit_kv_chunk(2)
            emit_vtr(2, nc.sync)
            ot2 = emit_slot(2)
            emit_kv_chunk(3)
            emit_vtr(3, nc.sync)
            emit_epi_b(1, *ot1)
            emit_epi_b(2, *ot2)
            for s, t, ysb in deferred_outs:
                nc.sync.dma_start(
                    out=out_d[s * 512 + t * 128:s * 512 + (t + 1) * 128, :],
                    in_=ysb[:],
                )
            deferred_outs.clear()
            ot3 = emit_slot(3)
            emit_epi_b(3, *ot3)

    nc.compile()
    return nc


_NC_CACHE = {}


def _make_in_maps(x, Wq, bq, Wk, bk, Wv, bv, Wo, bo):
    wkvA = np.concatenate([Wv, Wk], axis=1)                   # (512, 128) K hi
    wkvB = np.concatenate([Wk, Wv], axis=1)                   # (512, 128) K lo
    wo_aug = np.concatenate([Wo, (bv @ Wo + bo)[None, :]], axis=0).astype(ml_dtypes.bfloat16)
    wpack = np.zeros((128, 1280), np.float32)
    for j in range(4):
        wpack[:, j * 128:(j + 1) * 128] = wkvA[j * 128:(j + 1) * 128, :]
        wpack[:, 512 + j * 128:512 + (j + 1) * 128] = wkvB[j * 128:(j + 1) * 128, :]
        wpack[:, 1024 + j * H:1024 + (j + 1) * H] = Wq[j * 128:(j + 1) * 128, :]
    wpack = wpack.astype(ml_dtypes.bfloat16)

    fpack = np.zeros((128, 4), np.float32)
    fpack[64:, 0] = bk          # K^T on rows 64:128 (even half-chunks)
    fpack[0:64, 1] = bk         # K^T on rows 0:64  (odd half-chunks)
    fpack[0:64, 2] = bq
    fpack[64:, 2] = bq

    in_maps = []
    for c in range(8):
        b = c // 2
        blocks = BLOCKS_EVEN if c % 2 == 0 else BLOCKS_ODD
        xt = np.ascontiguousarray(x[b].T).astype(ml_dtypes.bfloat16)  # (512, 4096)
        qcols = np.concatenate(
            [np.arange(blk * QB, (blk + 1) * QB) for blk in blocks]
        )
        xtq = np.ascontiguousarray(xt[:, qcols])               # (512, 2048)
        thr = np.zeros((128, 16), np.float32)
        for s in range(4):
            for gm in range(4):
                thr[:, 4 * s + gm] = (NKT[s] - 8) * 128 - 512 * blocks[s] + 128 * gm
        in_maps.append({
            "xt": xt,
            "xtq": xtq,
            "wpack": wpack,
            "wo": wo_aug,
            "fpack": fpack,
            "thr": thr,
        })
    return in_maps


def kernel(x, Wq, bq, Wk, bk, Wv, bv, Wo, bo):
    global LAST_EXEC_TIME_NS, LAST_RESULTS
    x = np.asarray(x, dtype=np.float32)
    Wq, bq = np.asarray(Wq, np.float32), np.asarray(bq, np.float32)
    Wk, bk = np.asarray(Wk, np.float32), np.asarray(bk, np.float32)
    Wv, bv = np.asarray(Wv, np.float32), np.asarray(bv, np.float32)
    Wo, bo = np.asarray(Wo, np.float32), np.asarray(bo, np.float32)

    if "nc" not in _NC_CACHE:
        _NC_CACHE["nc"] = _build_nc()
    nc = _NC_CACHE["nc"]

    in_maps = _make_in_maps(x, Wq, bq, Wk, bk, Wv, bv, Wo, bo)

    trace = os.environ.get("KERNEL_TRACE", "1") == "1"
    if trace:
        trace = _install_ntff_hook()
    tmpdir = os.environ.get("KERNEL_TRACE_DIR") or None
    try:
        res = run_bass_kernel_spmd(
            nc, in_maps, core_ids=list(range(8)), trace=trace, tmpdir=tmpdir
        )
    except Exception:
        if not trace:
            raise
        res = run_bass_kernel_spmd(nc, in_maps, core_ids=list(range(8)), trace=False)
    LAST_EXEC_TIME_NS = res.exec_time_ns
    LAST_RESULTS = res

    out = np.empty((B, S, D), np.float32)
    for c in range(8):
        b = c // 2
        blocks = BLOCKS_EVEN if c % 2 == 0 else BLOCKS_ODD
        shard = res.results[c]["out"].astype(np.float32)
        for sidx, blk in enumerate(blocks):
            out[b, blk * QB:(blk + 1) * QB, :] = shard[sidx * QB:(sidx + 1) * QB, :]
    return out


# revision 35
# speedup vs baseline: 1.0140x; 1.0140x over previous
"""Causal single-head attention layer on 8 TRN2 NeuronCores.

Reference (per batch b):
  Q = x@Wq+bq; K = x@Wk+bk; V = x@Wv+bv        (S=4096, D=512, H=64)
  S = Q K^T / sqrt(S);  P = softmax(S + causal_mask);  out = (P V) @ Wo + bo

Sharding: 8 cores = 4 batches x 2 halves. Each core owns 4 query-blocks
of 512 rows of its batch in ASCENDING causal order: even cores take
blocks [0,3,4,7], odd take [1,2,5,6]. SPMD structural k-tile counts per
slot NKT=[8,16,24,32] cover both parities; over-structural k-tiles and
the causal boundary are killed by a multiplicative {0,1} mask generated
ON-CHIP from an fp16 iota ramp compared against per-core thresholds.

QK PE-array row-tiling (the big PE win vs the unpaired version): the QK
matmul contracts over H=64, which uses only half the 128-row array.
kvt alternates layout per 512-col half-chunk (even: V^T rows 0:64 /
K^T rows 64:128; odd: K^T 0:64 / V^T 64:128) and Q^T is duplicated to
both partition halves (the Q-projection pairs two M=64 matmuls on
disjoint column strips, so duplication costs no extra PE passes). Each
score group pairs one "hi" k-tile (8c+j) with one "lo" k-tile (8c+4+j);
the two QK matmuls land on disjoint 64-row groups (auto tile_position)
and execute CONCURRENTLY, near-halving QK PE time (269 -> 177 ns/MM
measured). The iota pattern [[-128,4],[-512,2],[1,512]] matches the
permuted (hi,lo) tail column order so one threshold per slot masks the
causal boundary. (An analogous split of the AV matmul into concurrent
K=64 halves measured SLOWER - more MM issues + psum merge work - and
column-tiled M=65 packing cannot host the ones-column denominator, so
AV stays one M=65 matmul per k-tile.)

Per group: S^T [128k, 1024] via the concurrent QK pair -> exp via ACT
(scale=1/64 folded) -> P^T bf16 -> multiplicative {0,1} mask (tail
groups only, DVE) -> AV accumulate otp[65,512] (V_aug carries a ones
column so the softmax denominator falls out of row 64; V natural layout
via XBAR transpose DMAs, source partition half alternating with the
kvt layout). QK(g+1) is emitted before AV(g) so the PE never waits on
the exp. Epilogue: all four denominator K=1 matmuls land in columns
0:4 of one psum tile, ONE reciprocal serves the slot (psum-buffer
churn between dn and y matmuls previously spaced the tail y-matmuls
~2us apart), y = ot^T @ [Wo; bv@Wo+bo] scaled by 1/denom -> bf16 out
DMA (host casts to f32). NOTE: skip_group_check=True on interleaved
same-tile accumulation chains mis-lowers start flags (NaN) - only the
Q-projection pair needs and tolerates it.

DMA schedule (measured ~150 GB/s aggregate effective during the
multi-queue lead-in; HWDGE rings are FIFO per issuing engine and a
descriptor is processed only after all prior ring entries fully
complete, ~2us HBM receipt each):
 - sync ring: fpack-equivalents ride scalar; xt[0:1024] as two 4-j-tile
   merged DMAs (a [128,4,512] AP view of xt^T - one dma_start each,
   ~600ns issue cost per dma_start on the sequencer), then vtr0, whose
   engine-block doubles as the GATE: the bulk loads emitted after it
   enter the rings only once the first wave has drained;
 - scalar ring: KV weights, xt[512:1024], Q weights, thr, wo - light,
   so the scalar engine reaches vtr-free exps early;
 - gpsimd SWDGE only carries slack-rich late loads (iota first, then
   xtq blocks 2-3, xt cols 2048:4096) - SWDGE descgen is slow (~0.6-
   1.5us/DMA) and its ring blocks the Q7, so nothing deadline-critical
   rides it. Slot 0-2 output DMAs are deferred and issued on sync just
   before slot 3; slot 3's fan out across scalar+sync.
 - PE p-state warmup (~3.4us HAM window at 1.2 GHz): 8 dependency-free
   matmuls burn the DMA lead-in so real matmuls run at 2.4 GHz.
"""

import os

os.environ.setdefault("MYCRO_LOCAL_CACHE", "1")

import numpy as np
import ml_dtypes

import concourse.mybir as mybir
import concourse.tile as tile
from concourse import bacc
from concourse.bass_utils import run_bass_kernel_spmd

F32 = mybir.dt.float32
F16 = mybir.dt.float16
BF16 = mybir.dt.bfloat16

B, S, D, H = 4, 4096, 512, 64
QB = 512
NKT = [8, 16, 24, 32]          # structural k-tiles per slot (ascending)
BLOCKS_EVEN = [0, 3, 4, 7]
BLOCKS_ODD = [1, 2, 5, 6]

LAST_EXEC_TIME_NS = None
LAST_RESULTS = None


def _install_ntff_hook():
    import sys
    import types
    try:
        from antenv.axon_hooks import get_axon_ntff_profile_hook  # noqa: F401
        return True
    except ImportError:
        pass
    try:
        import trn_agent_boot.trn_boot as _tb
        hook = _tb._ntff_profile_via_ctypes("/opt/axon/libaxon_pjrt.so")
        if hook is None:
            return False
        mod = types.ModuleType("antenv.axon_hooks")
        mod.get_axon_ntff_profile_hook = lambda: hook
        mod.set_axon_ntff_profile_hook = lambda h: None
        sys.modules["antenv.axon_hooks"] = mod
        return True
    except Exception:
        return False


def _build_nc():
    nc = bacc.Bacc(
        "TRN2",
        target_bir_lowering=False,
        debug=False,
        enable_asserts=False,
        num_devices=8,
    )

    xt_d = nc.dram_tensor("xt", [D, S], BF16, kind="ExternalInput")
    xtq_d = nc.dram_tensor("xtq", [D, 4 * QB], BF16, kind="ExternalInput")
    wpack_d = nc.dram_tensor("wpack", [128, 1280], BF16, kind="ExternalInput")
    wo_d = nc.dram_tensor("wo", [H + 1, D], BF16, kind="ExternalInput")
    fpack_d = nc.dram_tensor("fpack", [128, 4], F32, kind="ExternalInput")
    thr_d = nc.dram_tensor("thr", [128, 16], F32, kind="ExternalInput")
    out_d = nc.dram_tensor("out", [4 * QB, D], BF16, kind="ExternalOutput")

    with tile.TileContext(nc) as tc:
        with (
            tc.tile_pool(name="big", bufs=1) as big,
            tc.tile_pool(name="small", bufs=1) as small,
            tc.tile_pool(name="projps", bufs=2, space="PSUM") as projps,
            tc.tile_pool(name="stps", bufs=2, space="PSUM") as stps,
            tc.tile_pool(name="otps", bufs=2, space="PSUM") as otps,
            tc.tile_pool(name="ptp", bufs=4) as ptp,
            tc.tile_pool(name="epi", bufs=2) as epi,
        ):
            # ---- persistent SBUF ----
            xt4_sb = big.tile([128, 4, S], BF16, tag="xt4")
            xtq4_sb = big.tile([128, 4, 4 * QB], BF16, tag="xtq4")
            xt4_in = xt_d[:].rearrange("(j p) s -> p j s", p=128)
            xtq4_in = xtq_d[:].rearrange("(j p) s -> p j s", p=128)
            kvt_sb = big.tile([128, S], BF16, tag="kvt")      # per half-chunk: even V^T|K^T, odd K^T|V^T
            qtp_sb = big.tile([128, 4 * QB], BF16, tag="qtp")  # Q^T duplicated on both halves
            vaug_sb = big.tile([128, 32 * 80], BF16, tag="vaug")
            iota_sb = big.tile([128, 4, 2, 512], F16, tag="iota")
            mask_sb = big.tile([128, 4, 4096], BF16, tag="mask")
            wpack_sb = small.tile([128, 1280], BF16, tag="wpack")
            wo_sb = small.tile([H + 1, D], BF16, tag="wo")
            fpack_sb = small.tile([128, 4], F32, tag="fpack")
            thr_sb = small.tile([128, 16], F32, tag="thr")
            ones_sb = small.tile([128, 1], BF16, tag="ones")

            vaug3 = vaug_sb[:].rearrange("p (k c) -> p k c", c=80)
            bkv_hi = fpack_sb[:, 0:1]   # [0;bk]  (K^T on rows 64:128)
            bkv_lo = fpack_sb[:, 1:2]   # [bk;0]  (K^T on rows 0:64)
            bq_ap = fpack_sb[:, 2:3]    # bq duplicated both halves

            # ---- input DMAs ----
            # scalar HWDGE queue: KV weights, xt cols 512:1024, Q weights -
            # light, so the scalar engine reaches its first exp early.
            # sync: the critical first wave; the engine-blocking vtr0/vtr1
            # transposes act as natural gates for the bulk loads behind them.
            # gpsimd SWDGE: iota, then slack-rich late loads.
            nc.scalar.dma_start(out=wpack_sb[:, 0:1024], in_=wpack_d[:, 0:1024])
            nc.scalar.dma_start(out=fpack_sb[:], in_=fpack_d[:, :])
            nc.sync.dma_start(out=xt4_sb[:, :, 0:512], in_=xt4_in[:, :, 0:512])
            nc.sync.dma_start(out=xtq4_sb[:, :, 0:512], in_=xtq4_in[:, :, 0:512])
            nc.scalar.dma_start(out=xt4_sb[:, :, 512:1024], in_=xt4_in[:, :, 512:1024])
            nc.scalar.dma_start(out=wpack_sb[:, 1024:1280], in_=wpack_d[:, 1024:1280])
            nc.scalar.dma_start(out=thr_sb[:], in_=thr_d[:, :])
            nc.scalar.dma_start(out=wo_sb[:], in_=wo_d[:, :])
            # iota ramp v[p,a,b,j] = -128a - 512b + j - p  (f16 exact, |v|<2048)
            # column order matches (group, hi/lo, q) so kt_rel = a + 4b
            nc.gpsimd.iota(
                iota_sb[:], pattern=[[-128, 4], [-512, 2], [1, 512]], base=0,
                channel_multiplier=-1, allow_small_or_imprecise_dtypes=True,
            )

            def emit_xt(c0, c1, eng):
                eng.dma_start(out=xt4_sb[:, :, c0:c1], in_=xt4_in[:, :, c0:c1])

            def emit_xtq(c0, c1, eng):
                eng.dma_start(out=xtq4_sb[:, :, c0:c1], in_=xtq4_in[:, :, c0:c1])

            # PE p-state warmup: dependency-free matmuls burn the DMA lead-in
            warm_sb = small.tile([128, 512], BF16, tag="warm")
            nc.vector.memset(warm_sb[:], 0.0)

            def emit_warm(n):
                # one psum tile for all warm matmuls (they serialize on
                # has_written, which is fine - the point is PE activity)
                wp = projps.tile([128, 512], F32, tag="proj")
                for _ in range(n):
                    nc.tensor.matmul(
                        wp[:], lhsT=warm_sb[:, 0:128], rhs=warm_sb[:],
                        start=True, stop=True,
                    )

            emit_warm(9)
            nc.vector.memset(vaug3[:, :, 64:65], 1.0)
            nc.vector.memset(ones_sb[:], 1.0)

            def emit_mask(s):
                # keep-mask[p,c] = 1.0 where iota >= thr[s] else 0  (f16 in, bf16 out)
                nc.vector.tensor_scalar(
                    out=mask_sb[:, s, :],
                    in0=iota_sb[:].rearrange("p a b c -> p (a b c)"),
                    scalar1=thr_sb[:, 4 * s:4 * s + 1],
                    scalar2=None,
                    op0=mybir.AluOpType.is_ge,
                )

            def emit_kv_chunk(c):
                # seq cols [c*1024,(c+1)*1024) = k-tiles 8c..8c+7
                # half-chunk h=2c+half: h even -> [V^T;K^T] (wpack A), h odd -> [K^T;V^T] (B)
                for half in range(2):
                    h = 2 * c + half
                    col = c * 1024 + half * 512
                    woff = 0 if h % 2 == 0 else 512
                    kvp = projps.tile([128, 512], F32, tag="proj")
                    for j in range(4):
                        nc.tensor.matmul(
                            kvp[:],
                            lhsT=wpack_sb[:, woff + j * 128:woff + (j + 1) * 128],
                            rhs=xt4_sb[:, j, col:col + 512],
                            start=(j == 0),
                            stop=(j == 3),
                        )
                    nc.vector.tensor_scalar_add(
                        kvt_sb[:, col:col + 512], kvp[:],
                        bkv_hi if h % 2 == 0 else bkv_lo,
                    )

            def emit_vtr(c, eng):
                # V^T -> V natural via the XBAR transpose DMA, per 512-col piece.
                # V^T partition half alternates with the kvt layout.
                for p in range(2):
                    h = 2 * c + p
                    col = c * 1024 + p * 512
                    kt0 = 4 * h
                    src = kvt_sb[0:64, col:col + 512] if h % 2 == 0 \
                        else kvt_sb[64:128, col:col + 512]
                    eng.dma_start_transpose(
                        out=vaug3[:, kt0:kt0 + 4, 0:64], in_=src,
                    )

            def emit_q_block(blk):
                # Q^T duplicated to BOTH partition halves: two M=64 matmuls on
                # disjoint column strips run concurrently (col tiling) - the
                # duplication is free. lo -> qp[0:64], hi -> qp[64:128].
                qp = projps.tile([128, 512], F32, tag="proj")
                for j in range(4):
                    for lo in (True, False):
                        nc.tensor.matmul(
                            qp[0:64, :] if lo else qp[64:128, :],
                            lhsT=wpack_sb[:, 1024 + j * H:1024 + (j + 1) * H],
                            rhs=xtq4_sb[:, j, blk * 512:(blk + 1) * 512],
                            start=(j == 0),
                            stop=(j == 3),
                            skip_group_check=True,
                        )
                nc.vector.tensor_scalar_add(
                    qtp_sb[:, blk * 512:(blk + 1) * 512], qp[:], bq_ap
                )

            def emit_slot(s):
                nkt = NKT[s]
                ngrp = nkt // 2
                otp = otps.tile([H + 1, 512], F32, tag="otp")
                pts = {}

                def pair(g):
                    c8, j = divmod(g, 4)
                    return 8 * c8 + j, 8 * c8 + 4 + j  # (hi kt, lo kt)

                for g in range(ngrp + 1):
                    if g < ngrp:
                        ktH, ktL = pair(g)
                        stp = stps.tile([128, 1024], F32, tag="stp")
                        # concurrent row-tiled QK pair (disjoint 64-row groups)
                        nc.tensor.matmul(
                            stp[:, 0:512],
                            lhsT=kvt_sb[64:128, ktH * 128:(ktH + 1) * 128],
                            rhs=qtp_sb[64:128, s * 512:(s + 1) * 512],
                            start=True, stop=True,
                        )
                        nc.tensor.matmul(
                            stp[:, 512:1024],
                            lhsT=kvt_sb[0:64, ktL * 128:(ktL + 1) * 128],
                            rhs=qtp_sb[0:64, s * 512:(s + 1) * 512],
                            start=True, stop=True,
                        )
                        pt = ptp.tile([128, 1024], BF16, tag="pt")
                        nc.scalar.activation(
                            pt[:], stp[:], mybir.ActivationFunctionType.Exp,
                            scale=1.0 / 64.0,
                        )
                        if g >= ngrp - 4:
                            gm = g - (ngrp - 4)
                            nc.vector.tensor_mul(
                                pt[:],
                                pt[:],
                                mask_sb[:, s, gm * 1024:(gm + 1) * 1024],
                            )
                        pts[g] = pt
                    if g >= 1:
                        ptm = pts.pop(g - 1)
                        ktH, ktL = pair(g - 1)
                        for u, kt in ((0, ktH), (1, ktL)):
                            nc.tensor.matmul(
                                otp[:],
                                lhsT=vaug3[:, kt, 0:65],
                                rhs=ptm[:, u * 512:(u + 1) * 512],
                                start=(g == 1 and u == 0),
                                stop=(g == ngrp and u == 1),
                            )
                # epilogue A: stash ot (incl. denominator row 64) in bf16
                ot_sb = epi.tile([H + 1, 512], BF16, tag="ot_sb")
                dnrow = epi.tile([1, 512], BF16, tag="dnrow")
                if s == 3:
                    # kernel tail: halve the chain across both idle engines
                    nc.scalar.copy(dnrow[:], otp[H:H + 1, :])
                    nc.vector.tensor_copy(ot_sb[:, 0:256], otp[:, 0:256])
                    nc.scalar.copy(ot_sb[:, 256:512], otp[:, 256:512])
                else:
                    nc.vector.tensor_copy(ot_sb[:], otp[:])
                    nc.vector.tensor_copy(dnrow[:], otp[H:H + 1, :])
                return ot_sb, dnrow

            deferred_outs = []

            def emit_epi_b(s, ot_sb, dnrow):
                # the final slot's epilogue is the kernel tail: fan its scale
                # and output DMA across two engines/queues each. All four
                # denominator matmuls land in one psum tile (column per t) so
                # a single reciprocal serves the whole slot and the y matmuls
                # don't churn psum buffers against the dn chain.
                last = s == 3
                dnp = projps.tile([128, 512], F32, tag="proj")
                for t in range(4):
                    nc.tensor.matmul(
                        dnp[:, t:t + 1],
                        lhsT=dnrow[:, t * 128:(t + 1) * 128],
                        rhs=ones_sb[0:1, :],
                        start=True,
                        stop=True,
                    )
                recip = epi.tile([128, 4], F32, tag="recip")
                nc.vector.reciprocal(recip[:], dnp[:, 0:4])
                for t in range(4):
                    yp = projps.tile([128, 512], F32, tag="proj")
                    nc.tensor.matmul(
                        yp[:],
                        lhsT=ot_sb[:, t * 128:(t + 1) * 128],
                        rhs=wo_sb[:],
                        start=True,
                        stop=True,
                    )
                    ysb = epi.tile([128, 512], BF16, tag="ysb", bufs=14)
                    if last and t % 2 == 1:
                        nc.scalar.activation(
                            ysb[:], yp[:], mybir.ActivationFunctionType.Copy,
                            scale=recip[:, t:t + 1],
                        )
                    else:
                        nc.vector.tensor_scalar_mul(
                            ysb[:], yp[:], recip[:, t:t + 1]
                        )
                    if last:
                        out_eng = nc.scalar if t % 2 == 1 else nc.sync
                        out_eng.dma_start(
                            out=out_d[s * 512 + t * 128:s * 512 + (t + 1) * 128, :],
                            in_=ysb[:],
                        )
                    else:
                        # defer the out DMA: issuing it now would block the
                        # sync engine on the ysb semaphore while load DMAs
                        # still need issuing
                        deferred_outs.append((s, t, ysb))

            emit_kv_chunk(0)
            # vtr0 on sync doubles as the bulk-load gate: the sync engine
            # blocks here until kvt h0/h1 exist, so everything emitted after
            # enters the DMA rings only once the first wave has drained
            emit_vtr(0, nc.sync)
            emit_xt(1024, 2048, nc.sync)
            emit_xtq(512, 1024, nc.sync)
            emit_xtq(1024, 2048, nc.gpsimd)
            emit_xt(2048, 3072, nc.gpsimd)
            emit_xt(3072, 4096, nc.gpsimd)
            emit_q_block(0)
            emit_mask(0)
            emit_mask(1)
            ot0 = emit_slot(0)
            emit_q_block(1)
            emit_kv_chunk(1)
            emit_vtr(1, nc.sync)
            emit_mask(2)
            emit_mask(3)
            emit_epi_b(0, *ot0)
            ot1 = emit_slot(1)
            emit_kv_chunk(2)
            emit_q_block(2)
            emit_q_block(3)
            emit_vtr(2, nc.sync)
            ot2 = emit_slot(2)
            emit_kv_chunk(3)
            emit_vtr(3, nc.sync)
            emit_epi_b(1, *ot1)
            emit_epi_b(2, *ot2)
            for s, t, ysb in deferred_outs:
                nc.sync.dma_start(
                    out=out_d[s * 512 + t * 128:s * 512 + (t + 1) * 128, :],
                    in_=ysb[:],
                )
            deferred_outs.clear()
            ot3 = emit_slot(3)
            emit_epi_b(3, *ot3)

    nc.compile()
    return nc


_NC_CACHE = {}


def _make_in_maps(x, Wq, bq, Wk, bk, Wv, bv, Wo, bo):
    wkvA = np.concatenate([Wv, Wk], axis=1)                   # (512, 128) K hi
    wkvB = np.concatenate([Wk, Wv], axis=1)                   # (512, 128) K lo
    wo_aug = np.concatenate([Wo, (bv @ Wo + bo)[None, :]], axis=0).astype(ml_dtypes.bfloat16)
    wpack = np.zeros((128, 1280), np.float32)
    for j in range(4):
        wpack[:, j * 128:(j + 1) * 128] = wkvA[j * 128:(j + 1) * 128, :]
        wpack[:, 512 + j * 128:512 + (j + 1) * 128] = wkvB[j * 128:(j + 1) * 128, :]
        wpack[:, 1024 + j * H:1024 + (j + 1) * H] = Wq[j * 128:(j + 1) * 128, :]
    wpack = wpack.astype(ml_dtypes.bfloat16)

    fpack = np.zeros((128, 4), np.float32)
    fpack[64:, 0] = bk          # K^T on rows 64:128 (even half-chunks)
    fpack[0:64, 1] = bk         # K^T on rows 0:64  (odd half-chunks)
    fpack[0:64, 2] = bq
    fpack[64:, 2] = bq

    in_maps = []
    for c in range(8):
        b = c // 2
        blocks = BLOCKS_EVEN if c % 2 == 0 else BLOCKS_ODD
        xt = np.ascontiguousarray(x[b].T).astype(ml_dtypes.bfloat16)  # (512, 4096)
        qcols = np.concatenate(
            [np.arange(blk * QB, (blk + 1) * QB) for blk in blocks]
        )
        xtq = np.ascontiguousarray(xt[:, qcols])               # (512, 2048)
        thr = np.zeros((128, 16), np.float32)
        for s in range(4):
            for gm in range(4):
                thr[:, 4 * s + gm] = (NKT[s] - 8) * 128 - 512 * blocks[s] + 128 * gm
        in_maps.append({
            "xt": xt,
            "xtq": xtq,
            "wpack": wpack,
            "wo": wo_aug,
            "fpack": fpack,
            "thr": thr,
        })
    return in_maps


def kernel(x, Wq, bq, Wk, bk, Wv, bv, Wo, bo):
    global LAST_EXEC_TIME_NS, LAST_RESULTS
    x = np.asarray(x, dtype=np.float32)
    Wq, bq = np.asarray(Wq, np.float32), np.asarray(bq, np.float32)
    Wk, bk = np.asarray(Wk, np.float32), np.asarray(bk, np.float32)
    Wv, bv = np.asarray(Wv, np.float32), np.asarray(bv, np.float32)
    Wo, bo = np.asarray(Wo, np.float32), np.asarray(bo, np.float32)

    if "nc" not in _NC_CACHE:
        _NC_CACHE["nc"] = _build_nc()
    nc = _NC_CACHE["nc"]

    in_maps = _make_in_maps(x, Wq, bq, Wk, bk, Wv, bv, Wo, bo)

    trace = os.environ.get("KERNEL_TRACE", "1") == "1"
    if trace:
        trace = _install_ntff_hook()
    tmpdir = os.environ.get("KERNEL_TRACE_DIR") or None
    try:
        res = run_bass_kernel_spmd(
            nc, in_maps, core_ids=list(range(8)), trace=trace, tmpdir=tmpdir
        )
    except Exception:
        if not trace:
            raise
        res = run_bass_kernel_spmd(nc, in_maps, core_ids=list(range(8)), trace=False)
    LAST_EXEC_TIME_NS = res.exec_time_ns
    LAST_RESULTS = res

    out = np.empty((B, S, D), np.float32)
    for c in range(8):
        b = c // 2
        blocks = BLOCKS_EVEN if c % 2 == 0 else BLOCKS_ODD
        shard = res.results[c]["out"].astype(np.float32)
        for sidx, blk in enumerate(blocks):
            out[b, blk * QB:(blk + 1) * QB, :] = shard[sidx * QB:(sidx + 1) * QB, :]
    return out


# revision 36
# speedup vs baseline: 1.0196x; 1.0055x over previous
"""Causal single-head attention layer on 8 TRN2 NeuronCores.

Reference (per batch b):
  Q = x@Wq+bq; K = x@Wk+bk; V = x@Wv+bv        (S=4096, D=512, H=64)
  S = Q K^T / sqrt(S);  P = softmax(S + causal_mask);  out = (P V) @ Wo + bo

Sharding: 8 cores = 4 batches x 2 halves. Each core owns 4 query-blocks
of 512 rows of its batch in ASCENDING causal order: even cores take
blocks [0,3,4,7], odd take [1,2,5,6]. SPMD structural k-tile counts per
slot NKT=[8,16,24,32] cover both parities; over-structural k-tiles and
the causal boundary are killed by a multiplicative {0,1} mask generated
ON-CHIP from an fp16 iota ramp compared against per-core thresholds.

QK PE-array row-tiling (the big PE win vs the unpaired version): the QK
matmul contracts over H=64, which uses only half the 128-row array.
kvt alternates layout per 512-col half-chunk (even: V^T rows 0:64 /
K^T rows 64:128; odd: K^T 0:64 / V^T 64:128) and Q^T is duplicated to
both partition halves (the Q-projection pairs two M=64 matmuls on
disjoint column strips, so duplication costs no extra PE passes). Each
score group pairs one "hi" k-tile (8c+j) with one "lo" k-tile (8c+4+j);
the two QK matmuls land on disjoint 64-row groups (auto tile_position)
and execute CONCURRENTLY, near-halving QK PE time (269 -> 177 ns/MM
measured). The iota pattern [[-128,4],[-512,2],[1,512]] matches the
permuted (hi,lo) tail column order so one threshold per slot masks the
causal boundary. (An analogous split of the AV matmul into concurrent
K=64 halves measured SLOWER - more MM issues + psum merge work - and
column-tiled M=65 packing cannot host the ones-column denominator, so
AV stays one M=65 matmul per k-tile.)

Per group: S^T [128k, 1024] via the concurrent QK pair -> exp via ACT
(scale=1/64 folded) -> P^T bf16 -> multiplicative {0,1} mask (tail
groups only, DVE) -> AV accumulate otp[65,512] (V_aug carries a ones
column so the softmax denominator falls out of row 64; V natural layout
via XBAR transpose DMAs, source partition half alternating with the
kvt layout). QK(g+1) is emitted before AV(g) so the PE never waits on
the exp. Epilogue: all four denominator K=1 matmuls land in columns
0:4 of one psum tile, ONE reciprocal serves the slot (psum-buffer
churn between dn and y matmuls previously spaced the tail y-matmuls
~2us apart), y = ot^T @ [Wo; bv@Wo+bo] scaled by 1/denom -> bf16 out
DMA (host casts to f32). NOTE: skip_group_check=True on interleaved
same-tile accumulation chains mis-lowers start flags (NaN) - only the
Q-projection pair needs and tolerates it.

DMA schedule (measured ~150 GB/s aggregate effective during the
multi-queue lead-in; HWDGE rings are FIFO per issuing engine and a
descriptor is processed only after all prior ring entries fully
complete, ~2us HBM receipt each):
 - sync ring: fpack-equivalents ride scalar; xt[0:1024] as two 4-j-tile
   merged DMAs (a [128,4,512] AP view of xt^T - one dma_start each,
   ~600ns issue cost per dma_start on the sequencer), then vtr0, whose
   engine-block doubles as the GATE: the bulk loads emitted after it
   enter the rings only once the first wave has drained;
 - scalar ring: KV weights, xt[512:1024], Q weights, thr, wo - light,
   so the scalar engine reaches vtr-free exps early;
 - gpsimd SWDGE only carries slack-rich late loads (iota first, then
   xtq blocks 2-3, xt cols 2048:4096) - SWDGE descgen is slow (~0.6-
   1.5us/DMA) and its ring blocks the Q7, so nothing deadline-critical
   rides it. Slot 0-2 output DMAs are deferred and issued on sync just
   before slot 3; slot 3's fan out across scalar+sync.
 - PE p-state warmup (~3.4us HAM window at 1.2 GHz): 8 dependency-free
   matmuls burn the DMA lead-in so real matmuls run at 2.4 GHz.
"""

import os

os.environ.setdefault("MYCRO_LOCAL_CACHE", "1")

import numpy as np
import ml_dtypes

import concourse.mybir as mybir
import concourse.tile as tile
from concourse import bacc
from concourse.bass_utils import run_bass_kernel_spmd

F32 = mybir.dt.float32
F16 = mybir.dt.float16
BF16 = mybir.dt.bfloat16

B, S, D, H = 4, 4096, 512, 64
QB = 512
NKT = [8, 16, 24, 32]          # structural k-tiles per slot (ascending)
BLOCKS_EVEN = [0, 3, 4, 7]
BLOCKS_ODD = [1, 2, 5, 6]

LAST_EXEC_TIME_NS = None
LAST_RESULTS = None


def _install_ntff_hook():
    import sys
    import types
    try:
        from antenv.axon_hooks import get_axon_ntff_profile_hook  # noqa: F401
        return True
    except ImportError:
        pass
    try:
        import trn_agent_boot.trn_boot as _tb
        hook = _tb._ntff_profile_via_ctypes("/opt/axon/libaxon_pjrt.so")
        if hook is None:
            return False
        mod = types.ModuleType("antenv.axon_hooks")
        mod.get_axon_ntff_profile_hook = lambda: hook
        mod.set_axon_ntff_profile_hook = lambda h: None
        sys.modules["antenv.axon_hooks"] = mod
        return True
    except Exception:
        return False


def _build_nc():
    nc = bacc.Bacc(
        "TRN2",
        target_bir_lowering=False,
        debug=False,
        enable_asserts=False,
        num_devices=8,
    )

    xt_d = nc.dram_tensor("xt", [D, S], BF16, kind="ExternalInput")
    xtq_d = nc.dram_tensor("xtq", [D, 4 * QB], BF16, kind="ExternalInput")
    wpack_d = nc.dram_tensor("wpack", [128, 1280], BF16, kind="ExternalInput")
    wo_d = nc.dram_tensor("wo", [H + 1, D], BF16, kind="ExternalInput")
    fpack_d = nc.dram_tensor("fpack", [128, 4], F32, kind="ExternalInput")
    thr_d = nc.dram_tensor("thr", [128, 16], F32, kind="ExternalInput")
    out_d = nc.dram_tensor("out", [4 * QB, D], BF16, kind="ExternalOutput")

    with tile.TileContext(nc) as tc:
        with (
            tc.tile_pool(name="big", bufs=1) as big,
            tc.tile_pool(name="small", bufs=1) as small,
            tc.tile_pool(name="projps", bufs=2, space="PSUM") as projps,
            tc.tile_pool(name="stps", bufs=2, space="PSUM") as stps,
            tc.tile_pool(name="otps", bufs=2, space="PSUM") as otps,
            tc.tile_pool(name="ptp", bufs=4) as ptp,
            tc.tile_pool(name="epi", bufs=2) as epi,
        ):
            # ---- persistent SBUF ----
            xt4_sb = big.tile([128, 4, S], BF16, tag="xt4")
            xtq4_sb = big.tile([128, 4, 4 * QB], BF16, tag="xtq4")
            xt4_in = xt_d[:].rearrange("(j p) s -> p j s", p=128)
            xtq4_in = xtq_d[:].rearrange("(j p) s -> p j s", p=128)
            kvt_sb = big.tile([128, S], BF16, tag="kvt")      # per half-chunk: even V^T|K^T, odd K^T|V^T
            qtp_sb = big.tile([128, 4 * QB], BF16, tag="qtp")  # Q^T duplicated on both halves
            vaug_sb = big.tile([128, 32 * 80], BF16, tag="vaug")
            iota_sb = big.tile([128, 4, 2, 512], F16, tag="iota")
            mask_sb = big.tile([128, 4, 4096], BF16, tag="mask")
            wpack_sb = small.tile([128, 1280], BF16, tag="wpack")
            wo_sb = small.tile([H + 1, D], BF16, tag="wo")
            fpack_sb = small.tile([128, 4], F32, tag="fpack")
            thr_sb = small.tile([128, 16], F32, tag="thr")
            ones_sb = small.tile([128, 1], BF16, tag="ones")

            vaug3 = vaug_sb[:].rearrange("p (k c) -> p k c", c=80)
            bkv_hi = fpack_sb[:, 0:1]   # [0;bk]  (K^T on rows 64:128)
            bkv_lo = fpack_sb[:, 1:2]   # [bk;0]  (K^T on rows 0:64)
            bq_ap = fpack_sb[:, 2:3]    # bq duplicated both halves

            # ---- input DMAs ----
            # scalar HWDGE queue: KV weights, xt cols 512:1024, Q weights -
            # light, so the scalar engine reaches its first exp early.
            # sync: the critical first wave; the engine-blocking vtr0/vtr1
            # transposes act as natural gates for the bulk loads behind them.
            # gpsimd SWDGE: iota, then slack-rich late loads.
            nc.scalar.dma_start(out=wpack_sb[:, 0:1024], in_=wpack_d[:, 0:1024])
            nc.scalar.dma_start(out=fpack_sb[:], in_=fpack_d[:, :])
            nc.sync.dma_start(out=xt4_sb[:, :, 0:512], in_=xt4_in[:, :, 0:512])
            nc.sync.dma_start(out=xtq4_sb[:, :, 0:512], in_=xtq4_in[:, :, 0:512])
            nc.scalar.dma_start(out=xt4_sb[:, :, 512:1024], in_=xt4_in[:, :, 512:1024])
            nc.scalar.dma_start(out=wpack_sb[:, 1024:1280], in_=wpack_d[:, 1024:1280])
            nc.scalar.dma_start(out=thr_sb[:], in_=thr_d[:, :])
            nc.scalar.dma_start(out=wo_sb[:], in_=wo_d[:, :])
            # iota ramp v[p,a,b,j] = -128a - 512b + j - p  (f16 exact, |v|<2048)
            # column order matches (group, hi/lo, q) so kt_rel = a + 4b
            nc.gpsimd.iota(
                iota_sb[:], pattern=[[-128, 4], [-512, 2], [1, 512]], base=0,
                channel_multiplier=-1, allow_small_or_imprecise_dtypes=True,
            )

            def emit_xt(c0, c1, eng):
                eng.dma_start(out=xt4_sb[:, :, c0:c1], in_=xt4_in[:, :, c0:c1])

            def emit_xtq(c0, c1, eng):
                eng.dma_start(out=xtq4_sb[:, :, c0:c1], in_=xtq4_in[:, :, c0:c1])

            # PE p-state warmup: dependency-free matmuls burn the DMA lead-in
            warm_sb = small.tile([128, 512], BF16, tag="warm")
            nc.vector.memset(warm_sb[:], 0.0)

            def emit_warm(n):
                for _ in range(n):
                    wp = projps.tile([128, 512], F32, tag="proj")
                    nc.tensor.matmul(
                        wp[:], lhsT=warm_sb[:, 0:128], rhs=warm_sb[:],
                        start=True, stop=True,
                    )

            emit_warm(8)
            nc.vector.memset(vaug3[:, :, 64:65], 1.0)
            nc.vector.memset(ones_sb[:], 1.0)

            def emit_mask(s):
                # keep-mask[p,c] = 1.0 where iota >= thr[s] else 0  (f16 in, bf16 out)
                nc.vector.tensor_scalar(
                    out=mask_sb[:, s, :],
                    in0=iota_sb[:].rearrange("p a b c -> p (a b c)"),
                    scalar1=thr_sb[:, 4 * s:4 * s + 1],
                    scalar2=None,
                    op0=mybir.AluOpType.is_ge,
                )

            def emit_kv_chunk(c):
                # seq cols [c*1024,(c+1)*1024) = k-tiles 8c..8c+7
                # half-chunk h=2c+half: h even -> [V^T;K^T] (wpack A), h odd -> [K^T;V^T] (B)
                for half in range(2):
                    h = 2 * c + half
                    col = c * 1024 + half * 512
                    woff = 0 if h % 2 == 0 else 512
                    kvp = projps.tile([128, 512], F32, tag="proj")
                    for j in range(4):
                        nc.tensor.matmul(
                            kvp[:],
                            lhsT=wpack_sb[:, woff + j * 128:woff + (j + 1) * 128],
                            rhs=xt4_sb[:, j, col:col + 512],
                            start=(j == 0),
                            stop=(j == 3),
                        )
                    nc.vector.tensor_scalar_add(
                        kvt_sb[:, col:col + 512], kvp[:],
                        bkv_hi if h % 2 == 0 else bkv_lo,
                    )

            def emit_vtr(c, eng):
                # V^T -> V natural via the XBAR transpose DMA, per 512-col piece.
                # V^T partition half alternates with the kvt layout.
                for p in range(2):
                    h = 2 * c + p
                    col = c * 1024 + p * 512
                    kt0 = 4 * h
                    src = kvt_sb[0:64, col:col + 512] if h % 2 == 0 \
                        else kvt_sb[64:128, col:col + 512]
                    eng.dma_start_transpose(
                        out=vaug3[:, kt0:kt0 + 4, 0:64], in_=src,
                    )

            def emit_q_block(blk):
                # Q^T duplicated to BOTH partition halves: two M=64 matmuls on
                # disjoint column strips run concurrently (col tiling) - the
                # duplication is free. lo -> qp[0:64], hi -> qp[64:128].
                qp = projps.tile([128, 512], F32, tag="proj")
                for j in range(4):
                    for lo in (True, False):
                        nc.tensor.matmul(
                            qp[0:64, :] if lo else qp[64:128, :],
                            lhsT=wpack_sb[:, 1024 + j * H:1024 + (j + 1) * H],
                            rhs=xtq4_sb[:, j, blk * 512:(blk + 1) * 512],
                            start=(j == 0),
                            stop=(j == 3),
                            skip_group_check=True,
                        )
                nc.vector.tensor_scalar_add(
                    qtp_sb[:, blk * 512:(blk + 1) * 512], qp[:], bq_ap
                )

            def emit_slot(s):
                nkt = NKT[s]
                ngrp = nkt // 2
                otp = otps.tile([H + 1, 512], F32, tag="otp")
                pts = {}

                def pair(g):
                    c8, j = divmod(g, 4)
                    return 8 * c8 + j, 8 * c8 + 4 + j  # (hi kt, lo kt)

                for g in range(ngrp + 1):
                    if g < ngrp:
                        ktH, ktL = pair(g)
                        stp = stps.tile([128, 1024], F32, tag="stp")
                        # concurrent row-tiled QK pair (disjoint 64-row groups)
                        nc.tensor.matmul(
                            stp[:, 0:512],
                            lhsT=kvt_sb[64:128, ktH * 128:(ktH + 1) * 128],
                            rhs=qtp_sb[64:128, s * 512:(s + 1) * 512],
                            start=True, stop=True,
                        )
                        nc.tensor.matmul(
                            stp[:, 512:1024],
                            lhsT=kvt_sb[0:64, ktL * 128:(ktL + 1) * 128],
                            rhs=qtp_sb[0:64, s * 512:(s + 1) * 512],
                            start=True, stop=True,
                        )
                        pt = ptp.tile([128, 1024], BF16, tag="pt")
                        nc.scalar.activation(
                            pt[:], stp[:], mybir.ActivationFunctionType.Exp,
                            scale=1.0 / 64.0,
                        )
                        if g >= ngrp - 4:
                            gm = g - (ngrp - 4)
                            nc.vector.tensor_mul(
                                pt[:],
                                pt[:],
                                mask_sb[:, s, gm * 1024:(gm + 1) * 1024],
                            )
                        pts[g] = pt
                    if g >= 1:
                        ptm = pts.pop(g - 1)
                        ktH, ktL = pair(g - 1)
                        for u, kt in ((0, ktH), (1, ktL)):
                            nc.tensor.matmul(
                                otp[:],
                                lhsT=vaug3[:, kt, 0:65],
                                rhs=ptm[:, u * 512:(u + 1) * 512],
                                start=(g == 1 and u == 0),
                                stop=(g == ngrp and u == 1),
                            )
                # epilogue A: stash ot (incl. denominator row 64) in bf16
                ot_sb = epi.tile([H + 1, 512], BF16, tag="ot_sb")
                dnrow = epi.tile([1, 512], BF16, tag="dnrow")
                if s == 3:
                    # kernel tail: halve the chain across both idle engines
                    nc.scalar.copy(dnrow[:], otp[H:H + 1, :])
                    nc.vector.tensor_copy(ot_sb[:, 0:256], otp[:, 0:256])
                    nc.scalar.copy(ot_sb[:, 256:512], otp[:, 256:512])
                else:
                    nc.vector.tensor_copy(ot_sb[:], otp[:])
                    nc.vector.tensor_copy(dnrow[:], otp[H:H + 1, :])
                return ot_sb, dnrow

            deferred_outs = []

            def emit_epi_b(s, ot_sb, dnrow):
                # the final slot's epilogue is the kernel tail: fan its scale
                # and output DMA across two engines/queues each. All four
                # denominator matmuls land in one psum tile (column per t) so
                # a single reciprocal serves the whole slot and the y matmuls
                # don't churn psum buffers against the dn chain.
                last = s == 3
                dnp = projps.tile([128, 512], F32, tag="proj")
                for t in range(4):
                    nc.tensor.matmul(
                        dnp[:, t:t + 1],
                        lhsT=dnrow[:, t * 128:(t + 1) * 128],
                        rhs=ones_sb[0:1, :],
                        start=True,
                        stop=True,
                    )
                recip = epi.tile([128, 4], F32, tag="recip")
                nc.vector.reciprocal(recip[:], dnp[:, 0:4])
                for t in range(4):
                    yp = projps.tile([128, 512], F32, tag="proj")
                    nc.tensor.matmul(
                        yp[:],
                        lhsT=ot_sb[:, t * 128:(t + 1) * 128],
                        rhs=wo_sb[:],
                        start=True,
                        stop=True,
                    )
                    ysb = epi.tile([128, 512], BF16, tag="ysb", bufs=14)
                    if last and t % 2 == 1:
                        nc.scalar.activation(
                            ysb[:], yp[:], mybir.ActivationFunctionType.Copy,
                            scale=recip[:, t:t + 1],
                        )
                    else:
                        nc.vector.tensor_scalar_mul(
                            ysb[:], yp[:], recip[:, t:t + 1]
                        )
                    if last:
                        out_eng = nc.scalar if t % 2 == 1 else nc.sync
                        out_eng.dma_start(
                            out=out_d[s * 512 + t * 128:s * 512 + (t + 1) * 128, :],
                            in_=ysb[:],
                        )
                    else:
                        # defer the out DMA: issuing it now would block the
                        # sync engine on the ysb semaphore while load DMAs
                        # still need issuing
                        deferred_outs.append((s, t, ysb))

            emit_kv_chunk(0)
            # vtr0 on sync doubles as the bulk-load gate: the sync engine
            # blocks here until kvt h0/h1 exist, so everything emitted after
            # enters the DMA rings only once the first wave has drained
            emit_vtr(0, nc.sync)
            emit_xtq(512, 1024, nc.sync)
            emit_xt(1024, 2048, nc.sync)
            emit_xtq(1024, 2048, nc.gpsimd)
            emit_xt(2048, 3072, nc.gpsimd)
            emit_xt(3072, 4096, nc.gpsimd)
            emit_q_block(0)
            emit_mask(0)
            emit_mask(1)
            ot0 = emit_slot(0)
            emit_q_block(1)
            emit_kv_chunk(1)
            emit_vtr(1, nc.sync)
            emit_mask(2)
            emit_mask(3)
            emit_epi_b(0, *ot0)
            ot1 = emit_slot(1)
            emit_kv_chunk(2)
            emit_q_block(2)
            emit_q_block(3)
            emit_vtr(2, nc.sync)
            ot2 = emit_slot(2)
            emit_kv_chunk(3)
            emit_vtr(3, nc.sync)
            emit_epi_b(1, *ot1)
            emit_epi_b(2, *ot2)
            for s, t, ysb in deferred_outs:
                nc.sync.dma_start(
                    out=out_d[s * 512 + t * 128:s * 512 + (t + 1) * 128, :],
                    in_=ysb[:],
                )
            deferred_outs.clear()
            ot3 = emit_slot(3)
            emit_epi_b(3, *ot3)

    nc.compile()
    return nc


_NC_CACHE = {}


def _make_in_maps(x, Wq, bq, Wk, bk, Wv, bv, Wo, bo):
    wkvA = np.concatenate([Wv, Wk], axis=1)                   # (512, 128) K hi
    wkvB = np.concatenate([Wk, Wv], axis=1)                   # (512, 128) K lo
    wo_aug = np.concatenate([Wo, (bv @ Wo + bo)[None, :]], axis=0).astype(ml_dtypes.bfloat16)
    wpack = np.zeros((128, 1280), np.float32)
    for j in range(4):
        wpack[:, j * 128:(j + 1) * 128] = wkvA[j * 128:(j + 1) * 128, :]
        wpack[:, 512 + j * 128:512 + (j + 1) * 128] = wkvB[j * 128:(j + 1) * 128, :]
        wpack[:, 1024 + j * H:1024 + (j + 1) * H] = Wq[j * 128:(j + 1) * 128, :]
    wpack = wpack.astype(ml_dtypes.bfloat16)

    fpack = np.zeros((128, 4), np.float32)
    fpack[64:, 0] = bk          # K^T on rows 64:128 (even half-chunks)
    fpack[0:64, 1] = bk         # K^T on rows 0:64  (odd half-chunks)
    fpack[0:64, 2] = bq
    fpack[64:, 2] = bq

    in_maps = []
    for c in range(8):
        b = c // 2
        blocks = BLOCKS_EVEN if c % 2 == 0 else BLOCKS_ODD
        xt = np.ascontiguousarray(x[b].T).astype(ml_dtypes.bfloat16)  # (512, 4096)
        qcols = np.concatenate(
            [np.arange(blk * QB, (blk + 1) * QB) for blk in blocks]
        )
        xtq = np.ascontiguousarray(xt[:, qcols])               # (512, 2048)
        thr = np.zeros((128, 16), np.float32)
        for s in range(4):
            for gm in range(4):
                thr[:, 4 * s + gm] = (NKT[s] - 8) * 128 - 512 * blocks[s] + 128 * gm
        in_maps.append({
            "xt": xt,
            "xtq": xtq,
            "wpack": wpack,
            "wo": wo_aug,
            "fpack": fpack,
            "thr": thr,
        })
    return in_maps


def kernel(x, Wq, bq, Wk, bk, Wv, bv, Wo, bo):
    global LAST_EXEC_TIME_NS, LAST_RESULTS
    x = np.asarray(x, dtype=np.float32)
    Wq, bq = np.asarray(Wq, np.float32), np.asarray(bq, np.float32)
    Wk, bk = np.asarray(Wk, np.float32), np.asarray(bk, np.float32)
    Wv, bv = np.asarray(Wv, np.float32), np.asarray(bv, np.float32)
    Wo, bo = np.asarray(Wo, np.float32), np.asarray(bo, np.float32)

    if "nc" not in _NC_CACHE:
        _NC_CACHE["nc"] = _build_nc()
    nc = _NC_CACHE["nc"]

    in_maps = _make_in_maps(x, Wq, bq, Wk, bk, Wv, bv, Wo, bo)

    trace = os.environ.get("KERNEL_TRACE", "1") == "1"
    if trace:
        trace = _install_ntff_hook()
    tmpdir = os.environ.get("KERNEL_TRACE_DIR") or None
    try:
        res = run_bass_kernel_spmd(
            nc, in_maps, core_ids=list(range(8)), trace=trace, tmpdir=tmpdir
        )
    except Exception:
        if not trace:
            raise
        res = run_bass_kernel_spmd(nc, in_maps, core_ids=list(range(8)), trace=False)
    LAST_EXEC_TIME_NS = res.exec_time_ns
    LAST_RESULTS = res

    out = np.empty((B, S, D), np.float32)
    for c in range(8):
        b = c // 2
        blocks = BLOCKS_EVEN if c % 2 == 0 else BLOCKS_ODD
        shard = res.results[c]["out"].astype(np.float32)
        for sidx, blk in enumerate(blocks):
            out[b, blk * QB:(blk + 1) * QB, :] = shard[sidx * QB:(sidx + 1) * QB, :]
    return out


# revision 38
# speedup vs baseline: 1.0262x; 1.0065x over previous
"""Causal single-head attention layer on 8 TRN2 NeuronCores.

Reference (per batch b):
  Q = x@Wq+bq; K = x@Wk+bk; V = x@Wv+bv        (S=4096, D=512, H=64)
  S = Q K^T / sqrt(S);  P = softmax(S + causal_mask);  out = (P V) @ Wo + bo

Sharding: 8 cores = 4 batches x 2 halves. Each core owns 4 query-blocks
of 512 rows of its batch in ASCENDING causal order: even cores take
blocks [0,3,4,7], odd take [1,2,5,6]. SPMD structural k-tile counts per
slot NKT=[8,16,24,32] cover both parities; over-structural k-tiles and
the causal boundary are killed by a multiplicative {0,1} mask generated
ON-CHIP from an fp16 iota ramp compared against per-core thresholds.

QK PE-array row-tiling (the big PE win vs the unpaired version): the QK
matmul contracts over H=64, which uses only half the 128-row array.
kvt alternates layout per 512-col half-chunk (even: V^T rows 0:64 /
K^T rows 64:128; odd: K^T 0:64 / V^T 64:128) and Q^T is duplicated to
both partition halves (the Q-projection pairs two M=64 matmuls on
disjoint column strips, so duplication costs no extra PE passes). Each
score group pairs one "hi" k-tile (8c+j) with one "lo" k-tile (8c+4+j);
the two QK matmuls land on disjoint 64-row groups (auto tile_position)
and execute CONCURRENTLY, near-halving QK PE time (269 -> 177 ns/MM
measured). The iota pattern [[-128,4],[-512,2],[1,512]] matches the
permuted (hi,lo) tail column order so one threshold per slot masks the
causal boundary. (An analogous split of the AV matmul into concurrent
K=64 halves measured SLOWER - more MM issues + psum merge work - and
column-tiled M=65 packing cannot host the ones-column denominator, so
AV stays one M=65 matmul per k-tile.)

Per group: S^T [128k, 1024] via the concurrent QK pair -> exp via ACT
(scale=1/64 folded) -> P^T bf16 -> multiplicative {0,1} mask (tail
groups only, DVE) -> AV accumulate otp[65,512] (V_aug carries a ones
column so the softmax denominator falls out of row 64; V natural layout
via XBAR transpose DMAs, source partition half alternating with the
kvt layout). QK(g+1) is emitted before AV(g) so the PE never waits on
the exp. Epilogue: all four denominator K=1 matmuls land in columns
0:4 of one psum tile, ONE reciprocal serves the slot (psum-buffer
churn between dn and y matmuls previously spaced the tail y-matmuls
~2us apart), y = ot^T @ [Wo; bv@Wo+bo] scaled by 1/denom -> bf16 out
DMA (host casts to f32). NOTE: skip_group_check=True on interleaved
same-tile accumulation chains mis-lowers start flags (NaN) - only the
Q-projection pair needs and tolerates it.

DMA schedule (measured ~150 GB/s aggregate effective during the
multi-queue lead-in; HWDGE rings are FIFO per issuing engine and a
descriptor is processed only after all prior ring entries fully
complete, ~2us HBM receipt each):
 - sync ring: fpack-equivalents ride scalar; xt[0:1024] as two 4-j-tile
   merged DMAs (a [128,4,512] AP view of xt^T - one dma_start each,
   ~600ns issue cost per dma_start on the sequencer), then vtr0, whose
   engine-block doubles as the GATE: the bulk loads emitted after it
   enter the rings only once the first wave has drained;
 - scalar ring: KV weights, xt[512:1024], Q weights, thr, wo - light,
   so the scalar engine reaches vtr-free exps early;
 - gpsimd SWDGE only carries slack-rich late loads (iota first, then
   xtq blocks 2-3, xt cols 2048:4096) - SWDGE descgen is slow (~0.6-
   1.5us/DMA) and its ring blocks the Q7, so nothing deadline-critical
   rides it. Slot 0-2 output DMAs are deferred and issued on sync just
   before slot 3; slot 3's fan out across scalar+sync.
 - PE p-state warmup (~3.4us HAM window at 1.2 GHz): 8 dependency-free
   matmuls burn the DMA lead-in so real matmuls run at 2.4 GHz.
"""

import os

os.environ.setdefault("MYCRO_LOCAL_CACHE", "1")

import numpy as np
import ml_dtypes

import concourse.mybir as mybir
import concourse.tile as tile
from concourse import bacc
from concourse.bass_utils import run_bass_kernel_spmd

F32 = mybir.dt.float32
F16 = mybir.dt.float16
BF16 = mybir.dt.bfloat16

B, S, D, H = 4, 4096, 512, 64
QB = 512
NKT = [8, 16, 24, 32]          # structural k-tiles per slot (ascending)
BLOCKS_EVEN = [0, 3, 4, 7]
BLOCKS_ODD = [1, 2, 5, 6]

LAST_EXEC_TIME_NS = None
LAST_RESULTS = None


def _install_ntff_hook():
    import sys
    import types
    try:
        from antenv.axon_hooks import get_axon_ntff_profile_hook  # noqa: F401
        return True
    except ImportError:
        pass
    try:
        import trn_agent_boot.trn_boot as _tb
        hook = _tb._ntff_profile_via_ctypes("/opt/axon/libaxon_pjrt.so")
        if hook is None:
            return False
        mod = types.ModuleType("antenv.axon_hooks")
        mod.get_axon_ntff_profile_hook = lambda: hook
        mod.set_axon_ntff_profile_hook = lambda h: None
        sys.modules["antenv.axon_hooks"] = mod
        return True
    except Exception:
        return False


def _build_nc():
    nc = bacc.Bacc(
        "TRN2",
        target_bir_lowering=False,
        debug=False,
        enable_asserts=False,
        num_devices=8,
    )

    xt_d = nc.dram_tensor("xt", [D, S], BF16, kind="ExternalInput")
    xtq_d = nc.dram_tensor("xtq", [D, 4 * QB], BF16, kind="ExternalInput")
    wpack_d = nc.dram_tensor("wpack", [128, 1280], BF16, kind="ExternalInput")
    wo_d = nc.dram_tensor("wo", [H + 1, D], BF16, kind="ExternalInput")
    fpack_d = nc.dram_tensor("fpack", [128, 4], F32, kind="ExternalInput")
    thr_d = nc.dram_tensor("thr", [128, 16], F32, kind="ExternalInput")
    out_d = nc.dram_tensor("out", [4 * QB, D], BF16, kind="ExternalOutput")

    with tile.TileContext(nc) as tc:
        with (
            tc.tile_pool(name="big", bufs=1) as big,
            tc.tile_pool(name="small", bufs=1) as small,
            tc.tile_pool(name="projps", bufs=2, space="PSUM") as projps,
            tc.tile_pool(name="stps", bufs=2, space="PSUM") as stps,
            tc.tile_pool(name="otps", bufs=2, space="PSUM") as otps,
            tc.tile_pool(name="ptp", bufs=4) as ptp,
            tc.tile_pool(name="epi", bufs=2) as epi,
        ):
            # ---- persistent SBUF ----
            xt4_sb = big.tile([128, 4, S], BF16, tag="xt4")
            xtq4_sb = big.tile([128, 4, 4 * QB], BF16, tag="xtq4")
            xt4_in = xt_d[:].rearrange("(j p) s -> p j s", p=128)
            xtq4_in = xtq_d[:].rearrange("(j p) s -> p j s", p=128)
            kvt_sb = big.tile([128, S], BF16, tag="kvt")      # per half-chunk: even V^T|K^T, odd K^T|V^T
            qtp_sb = big.tile([128, 4 * QB], BF16, tag="qtp")  # Q^T duplicated on both halves
            vaug_sb = big.tile([128, 32 * 80], BF16, tag="vaug")
            iota_sb = big.tile([128, 4, 2, 512], F16, tag="iota")
            mask_sb = big.tile([128, 4, 4096], BF16, tag="mask")
            wpack_sb = small.tile([128, 1280], BF16, tag="wpack")
            wo_sb = small.tile([H + 1, D], BF16, tag="wo")
            fpack_sb = small.tile([128, 4], F32, tag="fpack")
            thr_sb = small.tile([128, 16], F32, tag="thr")
            ones_sb = small.tile([128, 1], BF16, tag="ones")

            vaug3 = vaug_sb[:].rearrange("p (k c) -> p k c", c=80)
            bkv_hi = fpack_sb[:, 0:1]   # [0;bk]  (K^T on rows 64:128)
            bkv_lo = fpack_sb[:, 1:2]   # [bk;0]  (K^T on rows 0:64)
            bq_ap = fpack_sb[:, 2:3]    # bq duplicated both halves

            # ---- input DMAs ----
            # scalar HWDGE queue: KV weights, xt cols 512:1024, Q weights -
            # light, so the scalar engine reaches its first exp early.
            # sync: the critical first wave; the engine-blocking vtr0/vtr1
            # transposes act as natural gates for the bulk loads behind them.
            # gpsimd SWDGE: iota, then slack-rich late loads.
            nc.scalar.dma_start(out=wpack_sb[:, 0:1024], in_=wpack_d[:, 0:1024])
            nc.scalar.dma_start(out=fpack_sb[:], in_=fpack_d[:, :])
            nc.sync.dma_start(out=xt4_sb[:, :, 0:512], in_=xt4_in[:, :, 0:512])
            nc.sync.dma_start(out=xtq4_sb[:, :, 0:512], in_=xtq4_in[:, :, 0:512])
            nc.scalar.dma_start(out=xt4_sb[:, :, 512:1024], in_=xt4_in[:, :, 512:1024])
            nc.scalar.dma_start(out=wpack_sb[:, 1024:1280], in_=wpack_d[:, 1024:1280])
            nc.scalar.dma_start(out=thr_sb[:], in_=thr_d[:, :])
            nc.scalar.dma_start(out=wo_sb[:], in_=wo_d[:, :])
            # iota ramp v[p,a,b,j] = -128a - 512b + j - p  (f16 exact, |v|<2048)
            # column order matches (group, hi/lo, q) so kt_rel = a + 4b
            nc.gpsimd.iota(
                iota_sb[:], pattern=[[-128, 4], [-512, 2], [1, 512]], base=0,
                channel_multiplier=-1, allow_small_or_imprecise_dtypes=True,
            )

            def emit_xt(c0, c1, eng):
                eng.dma_start(out=xt4_sb[:, :, c0:c1], in_=xt4_in[:, :, c0:c1])

            def emit_xtq(c0, c1, eng):
                eng.dma_start(out=xtq4_sb[:, :, c0:c1], in_=xtq4_in[:, :, c0:c1])

            # PE p-state warmup: dependency-free matmuls burn the DMA lead-in
            warm_sb = small.tile([128, 512], BF16, tag="warm")
            nc.vector.memset(warm_sb[:], 0.0)

            def emit_warm(n):
                for _ in range(n):
                    wp = projps.tile([128, 512], F32, tag="proj")
                    nc.tensor.matmul(
                        wp[:], lhsT=warm_sb[:, 0:128], rhs=warm_sb[:],
                        start=True, stop=True,
                    )

            emit_warm(8)
            nc.vector.memset(vaug3[:, :, 64:65], 1.0)
            nc.vector.memset(ones_sb[:], 1.0)

            def emit_mask(s):
                # keep-mask[p,c] = 1.0 where iota >= thr[s] else 0  (f16 in, bf16 out)
                nc.vector.tensor_scalar(
                    out=mask_sb[:, s, :],
                    in0=iota_sb[:].rearrange("p a b c -> p (a b c)"),
                    scalar1=thr_sb[:, 4 * s:4 * s + 1],
                    scalar2=None,
                    op0=mybir.AluOpType.is_ge,
                )

            def emit_kv_chunk(c):
                # seq cols [c*1024,(c+1)*1024) = k-tiles 8c..8c+7
                # half-chunk h=2c+half: h even -> [V^T;K^T] (wpack A), h odd -> [K^T;V^T] (B)
                for half in range(2):
                    h = 2 * c + half
                    col = c * 1024 + half * 512
                    woff = 0 if h % 2 == 0 else 512
                    kvp = projps.tile([128, 512], F32, tag="proj")
                    for j in range(4):
                        nc.tensor.matmul(
                            kvp[:],
                            lhsT=wpack_sb[:, woff + j * 128:woff + (j + 1) * 128],
                            rhs=xt4_sb[:, j, col:col + 512],
                            start=(j == 0),
                            stop=(j == 3),
                        )
                    nc.vector.tensor_scalar_add(
                        kvt_sb[:, col:col + 512], kvp[:],
                        bkv_hi if h % 2 == 0 else bkv_lo,
                    )

            def emit_vtr(c, eng):
                # V^T -> V natural via the XBAR transpose DMA, per 512-col piece.
                # V^T partition half alternates with the kvt layout.
                for p in range(2):
                    h = 2 * c + p
                    col = c * 1024 + p * 512
                    kt0 = 4 * h
                    src = kvt_sb[0:64, col:col + 512] if h % 2 == 0 \
                        else kvt_sb[64:128, col:col + 512]
                    eng.dma_start_transpose(
                        out=vaug3[:, kt0:kt0 + 4, 0:64], in_=src,
                    )

            def emit_q_block(blk):
                # Q^T duplicated to BOTH partition halves: two M=64 matmuls on
                # disjoint column strips run concurrently (col tiling) - the
                # duplication is free. lo -> qp[0:64], hi -> qp[64:128].
                qp = projps.tile([128, 512], F32, tag="proj")
                for j in range(4):
                    for lo in (True, False):
                        nc.tensor.matmul(
                            qp[0:64, :] if lo else qp[64:128, :],
                            lhsT=wpack_sb[:, 1024 + j * H:1024 + (j + 1) * H],
                            rhs=xtq4_sb[:, j, blk * 512:(blk + 1) * 512],
                            start=(j == 0),
                            stop=(j == 3),
                            skip_group_check=True,
                        )
                nc.vector.tensor_scalar_add(
                    qtp_sb[:, blk * 512:(blk + 1) * 512], qp[:], bq_ap
                )

            def emit_slot(s):
                nkt = NKT[s]
                ngrp = nkt // 2
                otp = otps.tile([H + 1, 512], F32, tag="otp")
                pts = {}

                def pair(g):
                    c8, j = divmod(g, 4)
                    return 8 * c8 + j, 8 * c8 + 4 + j  # (hi kt, lo kt)

                for g in range(ngrp + 1):
                    if g < ngrp:
                        ktH, ktL = pair(g)
                        stp = stps.tile([128, 1024], F32, tag="stp")
                        # concurrent row-tiled QK pair (disjoint 64-row groups)
                        nc.tensor.matmul(
                            stp[:, 0:512],
                            lhsT=kvt_sb[64:128, ktH * 128:(ktH + 1) * 128],
                            rhs=qtp_sb[64:128, s * 512:(s + 1) * 512],
                            start=True, stop=True,
                        )
                        nc.tensor.matmul(
                            stp[:, 512:1024],
                            lhsT=kvt_sb[0:64, ktL * 128:(ktL + 1) * 128],
                            rhs=qtp_sb[0:64, s * 512:(s + 1) * 512],
                            start=True, stop=True,
                        )
                        pt = ptp.tile([128, 1024], BF16, tag="pt")
                        nc.scalar.activation(
                            pt[:], stp[:], mybir.ActivationFunctionType.Exp,
                            scale=1.0 / 64.0,
                        )
                        if g >= ngrp - 4:
                            gm = g - (ngrp - 4)
                            nc.vector.tensor_mul(
                                pt[:],
                                pt[:],
                                mask_sb[:, s, gm * 1024:(gm + 1) * 1024],
                            )
                        pts[g] = pt
                    if g >= 1:
                        ptm = pts.pop(g - 1)
                        ktH, ktL = pair(g - 1)
                        for u, kt in ((0, ktH), (1, ktL)):
                            nc.tensor.matmul(
                                otp[:],
                                lhsT=vaug3[:, kt, 0:65],
                                rhs=ptm[:, u * 512:(u + 1) * 512],
                                start=(g == 1 and u == 0),
                                stop=(g == ngrp and u == 1),
                            )
                # epilogue A: stash ot (incl. denominator row 64) in bf16
                ot_sb = epi.tile([H + 1, 512], BF16, tag="ot_sb")
                dnrow = epi.tile([1, 512], BF16, tag="dnrow")
                if s == 3:
                    # kernel tail: halve the chain across both idle engines
                    nc.scalar.copy(dnrow[:], otp[H:H + 1, :])
                    nc.vector.tensor_copy(ot_sb[:, 0:256], otp[:, 0:256])
                    nc.scalar.copy(ot_sb[:, 256:512], otp[:, 256:512])
                else:
                    nc.vector.tensor_copy(ot_sb[:], otp[:])
                    nc.vector.tensor_copy(dnrow[:], otp[H:H + 1, :])
                return ot_sb, dnrow

            deferred_outs = []

            def emit_epi_b(s, ot_sb, dnrow):
                # the final slot's epilogue is the kernel tail: fan its scale
                # and output DMA across two engines/queues each. All four
                # denominator matmuls land in one psum tile (column per t) so
                # a single reciprocal serves the whole slot and the y matmuls
                # don't churn psum buffers against the dn chain.
                last = s == 3
                dnp = projps.tile([128, 512], F32, tag="proj")
                for t in range(4):
                    nc.tensor.matmul(
                        dnp[:, t:t + 1],
                        lhsT=dnrow[:, t * 128:(t + 1) * 128],
                        rhs=ones_sb[0:1, :],
                        start=True,
                        stop=True,
                    )
                recip = epi.tile([128, 4], F32, tag="recip")
                nc.vector.reciprocal(recip[:], dnp[:, 0:4])
                for t in range(4):
                    yp = projps.tile([128, 512], F32, tag="proj")
                    nc.tensor.matmul(
                        yp[:],
                        lhsT=ot_sb[:, t * 128:(t + 1) * 128],
                        rhs=wo_sb[:],
                        start=True,
                        stop=True,
                    )
                    ysb = epi.tile([128, 512], BF16, tag="ysb", bufs=14)
                    if last and t % 2 == 1:
                        nc.scalar.activation(
                            ysb[:], yp[:], mybir.ActivationFunctionType.Copy,
                            scale=recip[:, t:t + 1],
                        )
                    else:
                        nc.vector.tensor_scalar_mul(
                            ysb[:], yp[:], recip[:, t:t + 1]
                        )
                    if last:
                        out_eng = nc.scalar if t % 2 == 1 else nc.sync
                        out_eng.dma_start(
                            out=out_d[s * 512 + t * 128:s * 512 + (t + 1) * 128, :],
                            in_=ysb[:],
                        )
                    else:
                        # defer the out DMA: issuing it now would block the
                        # sync engine on the ysb semaphore while load DMAs
                        # still need issuing
                        deferred_outs.append((s, t, ysb))

            emit_kv_chunk(0)
            # vtr0 on sync doubles as the bulk-load gate: the sync engine
            # blocks here until kvt h0/h1 exist, so everything emitted after
            # enters the DMA rings only once the first wave has drained
            emit_vtr(0, nc.sync)
            emit_xt(1024, 2048, nc.scalar)
            emit_xtq(512, 1024, nc.sync)
            emit_xtq(1024, 2048, nc.gpsimd)
            emit_xt(2048, 3072, nc.gpsimd)
            emit_xt(3072, 4096, nc.gpsimd)
            emit_q_block(0)
            emit_mask(0)
            emit_mask(1)
            ot0 = emit_slot(0)
            emit_q_block(1)
            emit_kv_chunk(1)
            emit_vtr(1, nc.sync)
            emit_mask(2)
            emit_mask(3)
            emit_epi_b(0, *ot0)
            ot1 = emit_slot(1)
            emit_kv_chunk(2)
            emit_q_block(2)
            emit_q_block(3)
            emit_vtr(2, nc.sync)
            ot2 = emit_slot(2)
            emit_kv_chunk(3)
            emit_vtr(3, nc.sync)
            emit_epi_b(1, *ot1)
            emit_epi_b(2, *ot2)
            for s, t, ysb in deferred_outs:
                nc.sync.dma_start(
                    out=out_d[s * 512 + t * 128:s * 512 + (t + 1) * 128, :],
                    in_=ysb[:],
                )
            deferred_outs.clear()
            ot3 = emit_slot(3)
            emit_epi_b(3, *ot3)

    nc.compile()
    return nc


_NC_CACHE = {}


def _make_in_maps(x, Wq, bq, Wk, bk, Wv, bv, Wo, bo):
    wkvA = np.concatenate([Wv, Wk], axis=1)                   # (512, 128) K hi
    wkvB = np.concatenate([Wk, Wv], axis=1)                   # (512, 128) K lo
    wo_aug = np.concatenate([Wo, (bv @ Wo + bo)[None, :]], axis=0).astype(ml_dtypes.bfloat16)
    wpack = np.zeros((128, 1280), np.float32)
    for j in range(4):
        wpack[:, j * 128:(j + 1) * 128] = wkvA[j * 128:(j + 1) * 128, :]
        wpack[:, 512 + j * 128:512 + (j + 1) * 128] = wkvB[j * 128:(j + 1) * 128, :]
        wpack[:, 1024 + j * H:1024 + (j + 1) * H] = Wq[j * 128:(j + 1) * 128, :]
    wpack = wpack.astype(ml_dtypes.bfloat16)

    fpack = np.zeros((128, 4), np.float32)
    fpack[64:, 0] = bk          # K^T on rows 64:128 (even half-chunks)
    fpack[0:64, 1] = bk         # K^T on rows 0:64  (odd half-chunks)
    fpack[0:64, 2] = bq
    fpack[64:, 2] = bq

    in_maps = []
    for c in range(8):
        b = c // 2
        blocks = BLOCKS_EVEN if c % 2 == 0 else BLOCKS_ODD
        xt = np.ascontiguousarray(x[b].T).astype(ml_dtypes.bfloat16)  # (512, 4096)
        qcols = np.concatenate(
            [np.arange(blk * QB, (blk + 1) * QB) for blk in blocks]
        )
        xtq = np.ascontiguousarray(xt[:, qcols])               # (512, 2048)
        thr = np.zeros((128, 16), np.float32)
        for s in range(4):
            for gm in range(4):
                thr[:, 4 * s + gm] = (NKT[s] - 8) * 128 - 512 * blocks[s] + 128 * gm
        in_maps.append({
            "xt": xt,
            "xtq": xtq,
            "wpack": wpack,
            "wo": wo_aug,
            "fpack": fpack,
            "thr": thr,
        })
    return in_maps


def kernel(x, Wq, bq, Wk, bk, Wv, bv, Wo, bo):
    global LAST_EXEC_TIME_NS, LAST_RESULTS
    x = np.asarray(x, dtype=np.float32)
    Wq, bq = np.asarray(Wq, np.float32), np.asarray(bq, np.float32)
    Wk, bk = np.asarray(Wk, np.float32), np.asarray(bk, np.float32)
    Wv, bv = np.asarray(Wv, np.float32), np.asarray(bv, np.float32)
    Wo, bo = np.asarray(Wo, np.float32), np.asarray(bo, np.float32)

    if "nc" not in _NC_CACHE:
        _NC_CACHE["nc"] = _build_nc()
    nc = _NC_CACHE["nc"]

    in_maps = _make_in_maps(x, Wq, bq, Wk, bk, Wv, bv, Wo, bo)

    trace = os.environ.get("KERNEL_TRACE", "1") == "1"
    if trace:
        trace = _install_ntff_hook()
    tmpdir = os.environ.get("KERNEL_TRACE_DIR") or None
    try:
        res = run_bass_kernel_spmd(
            nc, in_maps, core_ids=list(range(8)), trace=trace, tmpdir=tmpdir
        )
    except Exception:
        if not trace:
            raise
        res = run_bass_kernel_spmd(nc, in_maps, core_ids=list(range(8)), trace=False)
    LAST_EXEC_TIME_NS = res.exec_time_ns
    LAST_RESULTS = res

    out = np.empty((B, S, D), np.float32)
    for c in range(8):
        b = c // 2
        blocks = BLOCKS_EVEN if c % 2 == 0 else BLOCKS_ODD
        shard = res.results[c]["out"].astype(np.float32)
        for sidx, blk in enumerate(blocks):
            out[b, blk * QB:(blk + 1) * QB, :] = shard[sidx * QB:(sidx + 1) * QB, :]
    return out
